# revision 18
# baseline (speedup 1.0000x reference)
"""AngularLayer Trainium2 kernel: [500000, 63] -> [500000, 483].

Per row: 21 (x,y) landmarks -> 210 ordered-pair unit direction vectors
(clipped x/y components), appended to the input row.

Sharded batch-parallel over 8 NeuronCores (62500 rows/core), SPMD one graph.

Device computes ONLY the 420 tilt columns; the 63 passthrough columns are a
verbatim copy of the input, assembled on host. Per tile [parts x R rows]:

- DMA-in (sync/HWDGE): dense f32 rows [parts, R*63].
- ACT: gather the 42 (x,y) coords per row into packed fp16 pairs `pxy`
  (4B-aligned -> enables DVE 2x_1P packed mode downstream).
- DVE: 20 ragged pair-difference subs (minuend dense step-1 fp16,
  subtrahend one broadcast 32-bit (x,y) word -> both qualify for 2x),
  swap-add for the duplicated interleaved norm, tilt multiply.
- ACT: Square + Abs_reciprocal_sqrt on the duplicated layout (1 elem/cyc
  engine). rsqrt bias 1e-7 guards exact fp16 landmark collisions (avoids
  inf -> NaN); also makes the clip unnecessary (|tilt| <= 1 + 2^-10).
- DMA-out (gpsimd/SWDGE): fp16 tilts cast to f32 rows IN THE DMA.
- GPSIMD runs no tensor ops at all: its SBUF port is shared with DVE and
  any sustained GPSIMD tensor op starves both engines (measured).

Depth-4 software pipeline (load/gather@t, sub@t+1, sq@t+2, add@t+3,
rsq+mul+out@t+4): every dep is a full iteration old except rsq->mul,
which is hidden by queue position (rsq is ACT's first op, mul DVE's
third). The shallow depth + medium-sized tail tiles keep the output-DMA
stream close behind compute (a deep pipeline or big tail tiles bunch the
last tiles' output DMAs after the final muls and pay a long post-compute
drain; tiny tail tiles crawl on per-DMA fixed costs).

Measured on HW (8 cores, SPMD): 433 us; rel_err 4.9e-4 (baseline
f32-subtract kernel: 814 us, rel_err 2.2e-3). DMA roofline ~340 us.
"""

import os
from contextlib import ExitStack

import numpy as np

import concourse.bass as bass
import concourse.mybir as mybir
import concourse.tile as tile
from concourse import bacc
from concourse.bass_utils import run_bass_kernel_spmd

F32 = mybir.dt.float32
F16 = mybir.dt.float16
AF = mybir.ActivationFunctionType
ALU = mybir.AluOpType

N_CORES = 8
B_FULL = 500000
B_SHARD = B_FULL // N_CORES  # 62500
NLM = 21
NPAIR = 210
IN_C = 63
TILT_C = 2 * NPAIR  # 420
OUT_C = 483

ROWS_PER_PART = int(os.environ.get("ANGULAR_R", "16"))
RSQRT_EPS = 1e-7


def _plan_tiles(b_shard: int, r_main: int):
    """[(base, parts, rows_per_part)] covering b_shard rows exactly.

    The tail tapers to small tiles: when loads stop, the last `depth`
    in-flight tiles' output DMAs bunch up after the final muls, so the
    post-compute drain is proportional to the byte size of the last few
    tiles, not their count.
    """
    plan = []
    if b_shard == 62500 and r_main == 16:
        # medium-sized tail tiles: small enough to limit the post-compute
        # output bunch, big enough that per-DMA fixed costs don't dominate
        plan = [(128, 16)] * 25 + [(106, 10)] + [(128, 10)] * 8
    else:
        rows_left = b_shard
        while rows_left > 0:
            if rows_left >= 128 * r_main:
                plan.append((128, r_main))
                rows_left -= 128 * r_main
                continue
            for r in (r_main, 10, 5, 4, 2, 1):
                if rows_left % r == 0 and rows_left // r <= 128:
                    plan.append((rows_left // r, r))
                    rows_left = 0
                    break
            else:
                raise AssertionError(rows_left)
    tiles = []
    base = 0
    for parts, r in plan:
        tiles.append((base, parts, r))
        base += parts * r
    assert base == b_shard, (base, b_shard)
    return tiles


def _build_nc(b_shard: int, rows_per_part: int) -> bass.Bass:
    tiles = _plan_tiles(b_shard, rows_per_part)
    n_tiles = len(tiles)

    nc = bacc.Bacc("TRN2", target_bir_lowering=False, debug=False)
    inp = nc.dram_tensor("tensor", [b_shard, IN_C], F32, kind="ExternalInput")
    outp = nc.dram_tensor("out", [b_shard, TILT_C], F32, kind="ExternalOutput")

    # rsqrt epsilon bias: register as a const AP so activation(bias=...) works
    eps_t = nc.alloc_sbuf_tensor(f"const-float32-{RSQRT_EPS}", [128, 1], F32)
    nc.gpsimd.memset(eps_t.ap(), RSQRT_EPS)
    nc.const_aps.aps[(F32, RSQRT_EPS)] = eps_t.ap()
    nc.all_engine_barrier()

    with tile.TileContext(nc) as tc, ExitStack() as ctx:
        rawp = ctx.enter_context(tc.tile_pool(name="raw", bufs=2))
        pxyp = ctx.enter_context(tc.tile_pool(name="pxy", bufs=2))
        vxyp = ctx.enter_context(tc.tile_pool(name="vxy", bufs=4))
        sqp = ctx.enter_context(tc.tile_pool(name="sq", bufs=2))
        nsqp = ctx.enter_context(tc.tile_pool(name="nsq", bufs=2))
        rrp = ctx.enter_context(tc.tile_pool(name="rr", bufs=2))
        ttp = ctx.enter_context(tc.tile_pool(name="tt", bufs=5))

        st: dict = {}

        def st_load(t):
            base, parts, R = tiles[t]
            raw = rawp.tile([parts, R * IN_C], F32, tag="raw")
            src = inp[base:base + parts * R, :].rearrange(
                "(p r) c -> p (r c)", p=parts)
            nc.sync.dma_start(out=raw[:], in_=src.opt())
            st[t] = {"raw": raw}

        def st_gather(t):
            # gather (x,y) of each landmark into 4B-aligned packed fp16 pairs
            _, parts, R = tiles[t]
            raw = st[t]["raw"]
            raw4 = raw[:].rearrange("p (r l three) -> p r l three", l=NLM,
                                    three=3)[:, :, :, 0:2]
            pxy = pxyp.tile([parts, R * 2 * NLM], F16, tag="pxy")
            pxy4 = pxy[:].rearrange("p (r l two) -> p r l two", l=NLM, two=2)
            nc.scalar.activation(pxy4, raw4, AF.Copy)
            del st[t]["raw"]
            st[t]["pxy"] = pxy

        def st_sub(t):
            # pair differences: minuend dense fp16 step-1, subtrahend one
            # broadcast 32-bit (x,y) word -> DVE 2x packed mode
            _, parts, R = tiles[t]
            pxy = st[t]["pxy"]
            pxy4 = pxy[:].rearrange("p (r l two) -> p r l two", l=NLM, two=2)
            vxy = vxyp.tile([parts, R * TILT_C], F16, tag="vxy")
            vxy4 = vxy[:].rearrange("p (r q two) -> p r q two", q=NPAIR, two=2)
            pb = 0
            for i in range(NLM - 1):
                np_i = NLM - 1 - i
                minu = pxy4[:, :, i + 1:NLM, :]
                subt = pxy4[:, :, i:i + 1, :].broadcast_to(
                    (parts, R, np_i, 2))
                nc.vector.tensor_sub(vxy4[:, :, pb:pb + np_i, :], minu, subt)
                pb += np_i
            del st[t]["pxy"]
            st[t]["vxy"] = vxy

        def st_sq(t):
            _, parts, R = tiles[t]
            sq = sqp.tile([parts, R * TILT_C], F16, tag="sq")
            nc.scalar.activation(sq[:], st[t]["vxy"][:], AF.Square)
            st[t]["sq"] = sq

        def st_add(t):
            # duplicated-interleaved norm: nsqd[2q+c] = sq[2q] + sq[2q+1]
            _, parts, R = tiles[t]
            sqv = st[t]["sq"][:].rearrange("p (q two) -> p q two", two=2)
            nsqd = nsqp.tile([parts, R * TILT_C], F16, tag="nsq")
            nsqv = nsqd[:].rearrange("p (q two) -> p q two", two=2)
            nc.vector.tensor_add(nsqv, sqv, sqv[:, :, ::-1])
            del st[t]["sq"]
            st[t]["nsq"] = nsqd

        def st_rsq(t):
            _, parts, R = tiles[t]
            rrd = rrp.tile([parts, R * TILT_C], F16, tag="rr")
            nc.scalar.activation(rrd[:], st[t]["nsq"][:],
                                 AF.Abs_reciprocal_sqrt, bias=RSQRT_EPS)
            del st[t]["nsq"]
            st[t]["rr"] = rrd

        def st_mul(t):
            _, parts, R = tiles[t]
            tt = ttp.tile([parts, R * TILT_C], F16, tag="tt")
            nc.vector.tensor_mul(tt[:], st[t]["vxy"][:], st[t]["rr"][:])
            del st[t]["vxy"], st[t]["rr"]
            st[t]["tt"] = tt

        def st_out(t):
            # SWDGE cast-DMA: fp16 tilts in SBUF -> f32 rows in DRAM
            base, parts, R = tiles[t]
            dst = outp[base:base + parts * R, :].rearrange(
                "(p r) c -> p (r c)", p=parts)
            nc.gpsimd.dma_start(out=dst.opt(), in_=st[t]["tt"][:])
            del st[t]

        # tile t: load@t, gather@t (ACT tail), sub@t+1, sq@t+2, add@t+3,
        # rsq@t+4 (ACT head), mul+out@t+4. rsq(t) is ACT's first op of its
        # iteration and mul(t) is DVE's third, so the same-iteration
        # rsq->mul edge is hidden by queue position; every other dep is a
        # full iteration old. The shallow depth keeps the output-DMA
        # stream close behind compute (see _plan_tiles).
        for s in range(n_tiles + 5):
            if s < n_tiles:
                st_load(s)
            if 0 <= s - 1 < n_tiles:
                st_sub(s - 1)
            if 0 <= s - 3 < n_tiles:
                st_add(s - 3)
            if 0 <= s - 4 < n_tiles:
                st_rsq(s - 4)
                st_mul(s - 4)
                st_out(s - 4)
            if 0 <= s - 2 < n_tiles:
                st_sq(s - 2)
            if s < n_tiles:
                st_gather(s)

    nc.compile()
    return nc


_NC_CACHE: dict = {}


def _get_nc():
    key = (B_SHARD, ROWS_PER_PART)
    if key not in _NC_CACHE:
        _NC_CACHE[key] = _build_nc(B_SHARD, ROWS_PER_PART)
    return _NC_CACHE[key]


def kernel(tensor: np.ndarray) -> np.ndarray:
    tensor = np.ascontiguousarray(np.asarray(tensor, dtype=np.float32))
    assert tensor.shape == (B_FULL, IN_C), tensor.shape

    nc = _get_nc()
    in_maps = [
        {"tensor": tensor[c * B_SHARD:(c + 1) * B_SHARD]} for c in range(N_CORES)
    ]
    trace = os.environ.get("ANGULAR_TRACE", "0") == "1"
    res = run_bass_kernel_spmd(
        nc, in_maps, core_ids=list(range(N_CORES)), trace=trace
    )
    if trace:
        kernel.last_exec_time_ns = res.exec_time_ns
        kernel.last_results = res

    out = np.empty((B_FULL, OUT_C), dtype=np.float32)
    out[:, :IN_C] = tensor
    for c in range(N_CORES):
        out[c * B_SHARD:(c + 1) * B_SHARD, IN_C:] = res.results[c]["out"]
    return out


# revision 20
# speedup vs baseline: 1.0181x; 1.0181x over previous
"""AngularLayer Trainium2 kernel: [500000, 63] -> [500000, 483].

Per row: 21 (x,y) landmarks -> 210 ordered-pair unit direction vectors
(clipped x/y components), appended to the input row.

Sharded batch-parallel over 8 NeuronCores (62500 rows/core), SPMD one graph.

Device computes ONLY the 420 tilt columns; the 63 passthrough columns are a
verbatim copy of the input, assembled on host. Per tile [parts x R rows]:

- DMA-in (sync/HWDGE): dense f32 rows [parts, R*63].
- ACT: gather the 42 (x,y) coords per row into packed fp16 pairs `pxy`
  (4B-aligned -> enables DVE 2x_1P packed mode downstream).
- DVE: 20 ragged pair-difference subs (minuend dense step-1 fp16,
  subtrahend one broadcast 32-bit (x,y) word -> both qualify for 2x),
  swap-add for the duplicated interleaved norm, tilt multiply.
- ACT: Square + Abs_reciprocal_sqrt on the duplicated layout (1 elem/cyc
  engine). rsqrt bias 1e-7 guards exact fp16 landmark collisions (avoids
  inf -> NaN); also makes the clip unnecessary (|tilt| <= 1 + 2^-10).
- DMA-out (gpsimd/SWDGE): fp16 tilts cast to f32 rows IN THE DMA.
- GPSIMD runs no tensor ops at all: its SBUF port is shared with DVE and
  any sustained GPSIMD tensor op starves both engines (measured).

Depth-4 software pipeline (load/gather@t, sub@t+1, sq@t+2, add@t+3,
rsq+mul+out@t+4): every dep is a full iteration old except rsq->mul,
which is hidden by queue position (rsq is ACT's first op, mul DVE's
third). The shallow depth + medium-sized tail tiles keep the output-DMA
stream close behind compute (a deep pipeline or big tail tiles bunch the
last tiles' output DMAs after the final muls and pay a long post-compute
drain; tiny tail tiles crawl on per-DMA fixed costs).

Measured on HW (8 cores, SPMD): 427.8 us; rel_err 4.9e-4 (baseline
f32-subtract kernel: 814.5 us, rel_err 2.2e-3). DMA roofline ~340 us.
Tail tuning measured: [128,8]x10 tail = 427.8; [128,10]x8 = 431.8;
[128,8]x6 = 433.0; [128,8]x4+[128,4]x4-style tiny tiles crawl (52-130
GB/s cast-DMA) - keep >= 6720 B/partition per output DMA.
"""

import os
from contextlib import ExitStack

import numpy as np

import concourse.bass as bass
import concourse.mybir as mybir
import concourse.tile as tile
from concourse import bacc
from concourse.bass_utils import run_bass_kernel_spmd

F32 = mybir.dt.float32
F16 = mybir.dt.float16
AF = mybir.ActivationFunctionType
ALU = mybir.AluOpType

N_CORES = 8
B_FULL = 500000
B_SHARD = B_FULL // N_CORES  # 62500
NLM = 21
NPAIR = 210
IN_C = 63
TILT_C = 2 * NPAIR  # 420
OUT_C = 483

ROWS_PER_PART = int(os.environ.get("ANGULAR_R", "16"))
RSQRT_EPS = 1e-7


def _plan_tiles(b_shard: int, r_main: int):
    """[(base, parts, rows_per_part)] covering b_shard rows exactly.

    The tail tapers to small tiles: when loads stop, the last `depth`
    in-flight tiles' output DMAs bunch up after the final muls, so the
    post-compute drain is proportional to the byte size of the last few
    tiles, not their count.
    """
    plan = []
    if b_shard == 62500 and r_main == 16:
        # medium-sized tail tiles: small enough to limit the post-compute
        # output bunch, big enough that per-DMA fixed costs don't dominate
        plan = [(128, 16)] * 23 + [(128, 12)] * 2 + [(106, 10)] + [(128, 8)] * 11
    else:
        rows_left = b_shard
        while rows_left > 0:
            if rows_left >= 128 * r_main:
                plan.append((128, r_main))
                rows_left -= 128 * r_main
                continue
            for r in (r_main, 10, 5, 4, 2, 1):
                if rows_left % r == 0 and rows_left // r <= 128:
                    plan.append((rows_left // r, r))
                    rows_left = 0
                    break
            else:
                raise AssertionError(rows_left)
    tiles = []
    base = 0
    for parts, r in plan:
        tiles.append((base, parts, r))
        base += parts * r
    assert base == b_shard, (base, b_shard)
    return tiles


def _build_nc(b_shard: int, rows_per_part: int) -> bass.Bass:
    tiles = _plan_tiles(b_shard, rows_per_part)
    n_tiles = len(tiles)

    nc = bacc.Bacc("TRN2", target_bir_lowering=False, debug=False)
    inp = nc.dram_tensor("tensor", [b_shard, IN_C], F32, kind="ExternalInput")
    outp = nc.dram_tensor("out", [b_shard, TILT_C], F32, kind="ExternalOutput")

    # rsqrt epsilon bias: register as a const AP so activation(bias=...) works
    eps_t = nc.alloc_sbuf_tensor(f"const-float32-{RSQRT_EPS}", [128, 1], F32)
    nc.gpsimd.memset(eps_t.ap(), RSQRT_EPS)
    nc.const_aps.aps[(F32, RSQRT_EPS)] = eps_t.ap()
    nc.all_engine_barrier()

    with tile.TileContext(nc) as tc, ExitStack() as ctx:
        rawp = ctx.enter_context(tc.tile_pool(name="raw", bufs=2))
        pxyp = ctx.enter_context(tc.tile_pool(name="pxy", bufs=2))
        vxyp = ctx.enter_context(tc.tile_pool(name="vxy", bufs=4))
        sqp = ctx.enter_context(tc.tile_pool(name="sq", bufs=2))
        nsqp = ctx.enter_context(tc.tile_pool(name="nsq", bufs=2))
        rrp = ctx.enter_context(tc.tile_pool(name="rr", bufs=2))
        ttp = ctx.enter_context(tc.tile_pool(name="tt", bufs=5))

        st: dict = {}

        def st_load(t):
            base, parts, R = tiles[t]
            raw = rawp.tile([parts, R * IN_C], F32, tag="raw")
            src = inp[base:base + parts * R, :].rearrange(
                "(p r) c -> p (r c)", p=parts)
            nc.sync.dma_start(out=raw[:], in_=src.opt())
            st[t] = {"raw": raw}

        def st_gather(t):
            # gather (x,y) of each landmark into 4B-aligned packed fp16 pairs
            _, parts, R = tiles[t]
            raw = st[t]["raw"]
            raw4 = raw[:].rearrange("p (r l three) -> p r l three", l=NLM,
                                    three=3)[:, :, :, 0:2]
            pxy = pxyp.tile([parts, R * 2 * NLM], F16, tag="pxy")
            pxy4 = pxy[:].rearrange("p (r l two) -> p r l two", l=NLM, two=2)
            nc.scalar.activation(pxy4, raw4, AF.Copy)
            del st[t]["raw"]
            st[t]["pxy"] = pxy

        def st_sub(t):
            # pair differences: minuend dense fp16 step-1, subtrahend one
            # broadcast 32-bit (x,y) word -> DVE 2x packed mode
            _, parts, R = tiles[t]
            pxy = st[t]["pxy"]
            pxy4 = pxy[:].rearrange("p (r l two) -> p r l two", l=NLM, two=2)
            vxy = vxyp.tile([parts, R * TILT_C], F16, tag="vxy")
            vxy4 = vxy[:].rearrange("p (r q two) -> p r q two", q=NPAIR, two=2)
            pb = 0
            for i in range(NLM - 1):
                np_i = NLM - 1 - i
                minu = pxy4[:, :, i + 1:NLM, :]
                subt = pxy4[:, :, i:i + 1, :].broadcast_to(
                    (parts, R, np_i, 2))
                nc.vector.tensor_sub(vxy4[:, :, pb:pb + np_i, :], minu, subt)
                pb += np_i
            del st[t]["pxy"]
            st[t]["vxy"] = vxy

        def st_sq(t):
            _, parts, R = tiles[t]
            sq = sqp.tile([parts, R * TILT_C], F16, tag="sq")
            nc.scalar.activation(sq[:], st[t]["vxy"][:], AF.Square)
            st[t]["sq"] = sq

        def st_add(t):
            # duplicated-interleaved norm: nsqd[2q+c] = sq[2q] + sq[2q+1]
            _, parts, R = tiles[t]
            sqv = st[t]["sq"][:].rearrange("p (q two) -> p q two", two=2)
            nsqd = nsqp.tile([parts, R * TILT_C], F16, tag="nsq")
            nsqv = nsqd[:].rearrange("p (q two) -> p q two", two=2)
            nc.vector.tensor_add(nsqv, sqv, sqv[:, :, ::-1])
            del st[t]["sq"]
            st[t]["nsq"] = nsqd

        def st_rsq(t):
            _, parts, R = tiles[t]
            rrd = rrp.tile([parts, R * TILT_C], F16, tag="rr")
            nc.scalar.activation(rrd[:], st[t]["nsq"][:],
                                 AF.Abs_reciprocal_sqrt, bias=RSQRT_EPS)
            del st[t]["nsq"]
            st[t]["rr"] = rrd

        def st_mul(t):
            _, parts, R = tiles[t]
            tt = ttp.tile([parts, R * TILT_C], F16, tag="tt")
            nc.vector.tensor_mul(tt[:], st[t]["vxy"][:], st[t]["rr"][:])
            del st[t]["vxy"], st[t]["rr"]
            st[t]["tt"] = tt

        def st_out(t):
            # SWDGE cast-DMA: fp16 tilts in SBUF -> f32 rows in DRAM
            base, parts, R = tiles[t]
            dst = outp[base:base + parts * R, :].rearrange(
                "(p r) c -> p (r c)", p=parts)
            nc.gpsimd.dma_start(out=dst.opt(), in_=st[t]["tt"][:])
            del st[t]

        # tile t: load@t, gather@t (ACT tail), sub@t+1, sq@t+2, add@t+3,
        # rsq@t+4 (ACT head), mul+out@t+4. rsq(t) is ACT's first op of its
        # iteration and mul(t) is DVE's third, so the same-iteration
        # rsq->mul edge is hidden by queue position; every other dep is a
        # full iteration old. The shallow depth keeps the output-DMA
        # stream close behind compute (see _plan_tiles).
        for s in range(n_tiles + 5):
            if s < n_tiles:
                st_load(s)
            if 0 <= s - 1 < n_tiles:
                st_sub(s - 1)
            if 0 <= s - 3 < n_tiles:
                st_add(s - 3)
            if 0 <= s - 4 < n_tiles:
                st_rsq(s - 4)
                st_mul(s - 4)
                st_out(s - 4)
            if 0 <= s - 2 < n_tiles:
                st_sq(s - 2)
            if s < n_tiles:
                st_gather(s)

    nc.compile()
    return nc


_NC_CACHE: dict = {}


def _get_nc():
    key = (B_SHARD, ROWS_PER_PART)
    if key not in _NC_CACHE:
        _NC_CACHE[key] = _build_nc(B_SHARD, ROWS_PER_PART)
    return _NC_CACHE[key]


def kernel(tensor: np.ndarray) -> np.ndarray:
    tensor = np.ascontiguousarray(np.asarray(tensor, dtype=np.float32))
    assert tensor.shape == (B_FULL, IN_C), tensor.shape

    nc = _get_nc()
    in_maps = [
        {"tensor": tensor[c * B_SHARD:(c + 1) * B_SHARD]} for c in range(N_CORES)
    ]
    trace = os.environ.get("ANGULAR_TRACE", "0") == "1"
    res = run_bass_kernel_spmd(
        nc, in_maps, core_ids=list(range(N_CORES)), trace=trace
    )
    if trace:
        kernel.last_exec_time_ns = res.exec_time_ns
        kernel.last_results = res

    out = np.empty((B_FULL, OUT_C), dtype=np.float32)
    out[:, :IN_C] = tensor
    for c in range(N_CORES):
        out[c * B_SHARD:(c + 1) * B_SHARD, IN_C:] = res.results[c]["out"]
    return out


# revision 21
# speedup vs baseline: 1.0212x; 1.0030x over previous
"""AngularLayer Trainium2 kernel: [500000, 63] -> [500000, 483].

Per row: 21 (x,y) landmarks -> 210 ordered-pair unit direction vectors
(clipped x/y components), appended to the input row.

Sharded batch-parallel over 8 NeuronCores (62500 rows/core), SPMD one graph.

Device computes ONLY the 420 tilt columns; the 63 passthrough columns are a
verbatim copy of the input, assembled on host. Per tile [parts x R rows]:

- DMA-in (sync/HWDGE): dense f32 rows [parts, R*63].
- ACT: gather the 42 (x,y) coords per row into packed fp16 pairs `pxy`
  (4B-aligned -> enables DVE 2x_1P packed mode downstream).
- DVE: 20 ragged pair-difference subs (minuend dense step-1 fp16,
  subtrahend one broadcast 32-bit (x,y) word -> both qualify for 2x),
  swap-add for the duplicated interleaved norm, tilt multiply.
- ACT: Square + Abs_reciprocal_sqrt on the duplicated layout (1 elem/cyc
  engine). rsqrt bias 1e-7 guards exact fp16 landmark collisions (avoids
  inf -> NaN); also makes the clip unnecessary (|tilt| <= 1 + 2^-10).
- DMA-out (gpsimd/SWDGE): fp16 tilts cast to f32 rows IN THE DMA.
- GPSIMD runs no tensor ops at all: its SBUF port is shared with DVE and
  any sustained GPSIMD tensor op starves both engines (measured).

Depth-4 software pipeline (load/gather@t, sub@t+1, sq@t+2, add@t+3,
rsq+mul+out@t+4): every dep is a full iteration old except rsq->mul,
which is hidden by queue position (rsq is ACT's first op, mul DVE's
third). The shallow depth + medium-sized tail tiles keep the output-DMA
stream close behind compute (a deep pipeline or big tail tiles bunch the
last tiles' output DMAs after the final muls and pay a long post-compute
drain; tiny tail tiles crawl on per-DMA fixed costs).

Measured on HW (8 cores, SPMD): 427.8 us; rel_err 4.9e-4 (baseline
f32-subtract kernel: 814.5 us, rel_err 2.2e-3). DMA roofline ~340 us.
Tail tuning measured: [128,8]x10 tail = 427.8; [128,10]x8 = 431.8;
[128,8]x6 = 433.0; [128,8]x4+[128,4]x4-style tiny tiles crawl (52-130
GB/s cast-DMA) - keep >= 6720 B/partition per output DMA.
"""

import os
from contextlib import ExitStack

import numpy as np

import concourse.bass as bass
import concourse.mybir as mybir
import concourse.tile as tile
from concourse import bacc
from concourse.bass_utils import run_bass_kernel_spmd

F32 = mybir.dt.float32
F16 = mybir.dt.float16
AF = mybir.ActivationFunctionType
ALU = mybir.AluOpType

N_CORES = 8
B_FULL = 500000
B_SHARD = B_FULL // N_CORES  # 62500
NLM = 21
NPAIR = 210
IN_C = 63
TILT_C = 2 * NPAIR  # 420
OUT_C = 483

ROWS_PER_PART = int(os.environ.get("ANGULAR_R", "16"))
RSQRT_EPS = 1e-7


def _plan_tiles(b_shard: int, r_main: int):
    """[(base, parts, rows_per_part)] covering b_shard rows exactly.

    The tail tapers to small tiles: when loads stop, the last `depth`
    in-flight tiles' output DMAs bunch up after the final muls, so the
    post-compute drain is proportional to the byte size of the last few
    tiles, not their count.
    """
    plan = []
    if b_shard == 62500 and r_main == 16:
        # medium-sized tail tiles: small enough to limit the post-compute
        # output bunch, big enough that per-DMA fixed costs don't dominate
        plan = ([(128, 16)] * 23 + [(106, 10)] + [(128, 14)] * 2
                + [(128, 12)] + [(128, 8)] * 9)
    else:
        rows_left = b_shard
        while rows_left > 0:
            if rows_left >= 128 * r_main:
                plan.append((128, r_main))
                rows_left -= 128 * r_main
                continue
            for r in (r_main, 10, 5, 4, 2, 1):
                if rows_left % r == 0 and rows_left // r <= 128:
                    plan.append((rows_left // r, r))
                    rows_left = 0
                    break
            else:
                raise AssertionError(rows_left)
    tiles = []
    base = 0
    for parts, r in plan:
        tiles.append((base, parts, r))
        base += parts * r
    assert base == b_shard, (base, b_shard)
    return tiles


def _build_nc(b_shard: int, rows_per_part: int) -> bass.Bass:
    tiles = _plan_tiles(b_shard, rows_per_part)
    n_tiles = len(tiles)

    nc = bacc.Bacc("TRN2", target_bir_lowering=False, debug=False)
    inp = nc.dram_tensor("tensor", [b_shard, IN_C], F32, kind="ExternalInput")
    outp = nc.dram_tensor("out", [b_shard, TILT_C], F32, kind="ExternalOutput")

    # rsqrt epsilon bias: register as a const AP so activation(bias=...) works
    eps_t = nc.alloc_sbuf_tensor(f"const-float32-{RSQRT_EPS}", [128, 1], F32)
    nc.gpsimd.memset(eps_t.ap(), RSQRT_EPS)
    nc.const_aps.aps[(F32, RSQRT_EPS)] = eps_t.ap()
    nc.all_engine_barrier()

    with tile.TileContext(nc) as tc, ExitStack() as ctx:
        rawp = ctx.enter_context(tc.tile_pool(name="raw", bufs=2))
        pxyp = ctx.enter_context(tc.tile_pool(name="pxy", bufs=2))
        vxyp = ctx.enter_context(tc.tile_pool(name="vxy", bufs=4))
        sqp = ctx.enter_context(tc.tile_pool(name="sq", bufs=2))
        nsqp = ctx.enter_context(tc.tile_pool(name="nsq", bufs=2))
        rrp = ctx.enter_context(tc.tile_pool(name="rr", bufs=2))
        ttp = ctx.enter_context(tc.tile_pool(name="tt", bufs=5))

        st: dict = {}

        def st_load(t):
            base, parts, R = tiles[t]
            raw = rawp.tile([parts, R * IN_C], F32, tag="raw")
            src = inp[base:base + parts * R, :].rearrange(
                "(p r) c -> p (r c)", p=parts)
            nc.sync.dma_start(out=raw[:], in_=src.opt())
            st[t] = {"raw": raw}

        def st_gather(t):
            # gather (x,y) of each landmark into 4B-aligned packed fp16 pairs
            _, parts, R = tiles[t]
            raw = st[t]["raw"]
            raw4 = raw[:].rearrange("p (r l three) -> p r l three", l=NLM,
                                    three=3)[:, :, :, 0:2]
            pxy = pxyp.tile([parts, R * 2 * NLM], F16, tag="pxy")
            pxy4 = pxy[:].rearrange("p (r l two) -> p r l two", l=NLM, two=2)
            nc.scalar.activation(pxy4, raw4, AF.Copy)
            del st[t]["raw"]
            st[t]["pxy"] = pxy

        def st_sub(t):
            # pair differences: minuend dense fp16 step-1, subtrahend one
            # broadcast 32-bit (x,y) word -> DVE 2x packed mode
            _, parts, R = tiles[t]
            pxy = st[t]["pxy"]
            pxy4 = pxy[:].rearrange("p (r l two) -> p r l two", l=NLM, two=2)
            vxy = vxyp.tile([parts, R * TILT_C], F16, tag="vxy")
            vxy4 = vxy[:].rearrange("p (r q two) -> p r q two", q=NPAIR, two=2)
            pb = 0
            for i in range(NLM - 1):
                np_i = NLM - 1 - i
                minu = pxy4[:, :, i + 1:NLM, :]
                subt = pxy4[:, :, i:i + 1, :].broadcast_to(
                    (parts, R, np_i, 2))
                nc.vector.tensor_sub(vxy4[:, :, pb:pb + np_i, :], minu, subt)
                pb += np_i
            del st[t]["pxy"]
            st[t]["vxy"] = vxy

        def st_sq(t):
            _, parts, R = tiles[t]
            sq = sqp.tile([parts, R * TILT_C], F16, tag="sq")
            nc.scalar.activation(sq[:], st[t]["vxy"][:], AF.Square)
            st[t]["sq"] = sq

        def st_add(t):
            # duplicated-interleaved norm: nsqd[2q+c] = sq[2q] + sq[2q+1]
            _, parts, R = tiles[t]
            sqv = st[t]["sq"][:].rearrange("p (q two) -> p q two", two=2)
            nsqd = nsqp.tile([parts, R * TILT_C], F16, tag="nsq")
            nsqv = nsqd[:].rearrange("p (q two) -> p q two", two=2)
            nc.vector.tensor_add(nsqv, sqv, sqv[:, :, ::-1])
            del st[t]["sq"]
            st[t]["nsq"] = nsqd

        def st_rsq(t):
            _, parts, R = tiles[t]
            rrd = rrp.tile([parts, R * TILT_C], F16, tag="rr")
            nc.scalar.activation(rrd[:], st[t]["nsq"][:],
                                 AF.Abs_reciprocal_sqrt, bias=RSQRT_EPS)
            del st[t]["nsq"]
            st[t]["rr"] = rrd

        def st_mul(t):
            _, parts, R = tiles[t]
            tt = ttp.tile([parts, R * TILT_C], F16, tag="tt")
            nc.vector.tensor_mul(tt[:], st[t]["vxy"][:], st[t]["rr"][:])
            del st[t]["vxy"], st[t]["rr"]
            st[t]["tt"] = tt

        def st_out(t):
            # SWDGE cast-DMA: fp16 tilts in SBUF -> f32 rows in DRAM
            base, parts, R = tiles[t]
            dst = outp[base:base + parts * R, :].rearrange(
                "(p r) c -> p (r c)", p=parts)
            nc.gpsimd.dma_start(out=dst.opt(), in_=st[t]["tt"][:])
            del st[t]

        # tile t: load@t, gather@t (ACT tail), sub@t+1, sq@t+2, add@t+3,
        # rsq@t+4 (ACT head), mul+out@t+4. rsq(t) is ACT's first op of its
        # iteration and mul(t) is DVE's third, so the same-iteration
        # rsq->mul edge is hidden by queue position; every other dep is a
        # full iteration old. The shallow depth keeps the output-DMA
        # stream close behind compute (see _plan_tiles).
        for s in range(n_tiles + 5):
            if s < n_tiles:
                st_load(s)
            if 0 <= s - 1 < n_tiles:
                st_sub(s - 1)
            if 0 <= s - 3 < n_tiles:
                st_add(s - 3)
            if 0 <= s - 4 < n_tiles:
                st_rsq(s - 4)
                st_mul(s - 4)
                st_out(s - 4)
            if 0 <= s - 2 < n_tiles:
                st_sq(s - 2)
            if s < n_tiles:
                st_gather(s)

    nc.compile()
    return nc


_NC_CACHE: dict = {}


def _get_nc():
    key = (B_SHARD, ROWS_PER_PART)
    if key not in _NC_CACHE:
        _NC_CACHE[key] = _build_nc(B_SHARD, ROWS_PER_PART)
    return _NC_CACHE[key]


def kernel(tensor: np.ndarray) -> np.ndarray:
    tensor = np.ascontiguousarray(np.asarray(tensor, dtype=np.float32))
    assert tensor.shape == (B_FULL, IN_C), tensor.shape

    nc = _get_nc()
    in_maps = [
        {"tensor": tensor[c * B_SHARD:(c + 1) * B_SHARD]} for c in range(N_CORES)
    ]
    trace = os.environ.get("ANGULAR_TRACE", "0") == "1"
    res = run_bass_kernel_spmd(
        nc, in_maps, core_ids=list(range(N_CORES)), trace=trace
    )
    if trace:
        kernel.last_exec_time_ns = res.exec_time_ns
        kernel.last_results = res

    out = np.empty((B_FULL, OUT_C), dtype=np.float32)
    out[:, :IN_C] = tensor
    for c in range(N_CORES):
        out[c * B_SHARD:(c + 1) * B_SHARD, IN_C:] = res.results[c]["out"]
    return out


# revision 22
# speedup vs baseline: 1.0317x; 1.0103x over previous
"""AngularLayer Trainium2 kernel: [500000, 63] -> [500000, 483].

Per row: 21 (x,y) landmarks -> 210 ordered-pair unit direction vectors
(clipped x/y components), appended to the input row.

Sharded batch-parallel over 8 NeuronCores (62500 rows/core), SPMD one graph.

Device computes ONLY the 420 tilt columns; the 63 passthrough columns are a
verbatim copy of the input, assembled on host. Per tile [parts x R rows]:

- DMA-in (sync/HWDGE): dense f32 rows [parts, R*63].
- ACT: gather the 42 (x,y) coords per row into packed fp16 pairs `pxy`
  (4B-aligned -> enables DVE 2x_1P packed mode downstream).
- DVE: 20 ragged pair-difference subs (minuend dense step-1 fp16,
  subtrahend one broadcast 32-bit (x,y) word -> both qualify for 2x),
  swap-add for the duplicated interleaved norm, tilt multiply.
- ACT: Square + Abs_reciprocal_sqrt on the duplicated layout (1 elem/cyc
  engine). rsqrt bias 1e-7 guards exact fp16 landmark collisions (avoids
  inf -> NaN); also makes the clip unnecessary (|tilt| <= 1 + 2^-10).
- DMA-out (gpsimd/SWDGE): fp16 tilts cast to f32 rows IN THE DMA.
- GPSIMD runs no tensor ops at all: its SBUF port is shared with DVE and
  any sustained GPSIMD tensor op starves both engines (measured).

Depth-4 software pipeline (load/gather@t, sub@t+1, sq@t+2, add@t+3,
rsq+mul+out@t+4): every dep is a full iteration old except rsq->mul,
which is hidden by queue position (rsq is ACT's first op, mul DVE's
third). The shallow depth + medium-sized tail tiles keep the output-DMA
stream close behind compute (a deep pipeline or big tail tiles bunch the
last tiles' output DMAs after the final muls and pay a long post-compute
drain; tiny tail tiles crawl on per-DMA fixed costs).

Measured on HW (8 cores, SPMD): 427.8 us; rel_err 4.9e-4 (baseline
f32-subtract kernel: 814.5 us, rel_err 2.2e-3). DMA roofline ~340 us.
Tail tuning measured: [128,8]x10 tail = 427.8; [128,10]x8 = 431.8;
[128,8]x6 = 433.0; [128,8]x4+[128,4]x4-style tiny tiles crawl (52-130
GB/s cast-DMA) - keep >= 6720 B/partition per output DMA.
"""

import os
from contextlib import ExitStack

import numpy as np

import concourse.bass as bass
import concourse.mybir as mybir
import concourse.tile as tile
from concourse import bacc
from concourse.bass_utils import run_bass_kernel_spmd

F32 = mybir.dt.float32
F16 = mybir.dt.float16
AF = mybir.ActivationFunctionType
ALU = mybir.AluOpType

N_CORES = 8
B_FULL = 500000
B_SHARD = B_FULL // N_CORES  # 62500
NLM = 21
NPAIR = 210
IN_C = 63
TILT_C = 2 * NPAIR  # 420
OUT_C = 483

ROWS_PER_PART = int(os.environ.get("ANGULAR_R", "16"))
RSQRT_EPS = 1e-7


def _plan_tiles(b_shard: int, r_main: int):
    """[(base, parts, rows_per_part)] covering b_shard rows exactly.

    The tail tapers to small tiles: when loads stop, the last `depth`
    in-flight tiles' output DMAs bunch up after the final muls, so the
    post-compute drain is proportional to the byte size of the last few
    tiles, not their count.
    """
    plan = []
    if b_shard == 62500 and r_main == 16:
        # medium-sized tail tiles: small enough to limit the post-compute
        # output bunch, big enough that per-DMA fixed costs don't dominate
        plan = ([(128, 16)] * 21 + [(106, 10)] + [(128, 14)] * 4
                + [(128, 8)] * 11)
    else:
        rows_left = b_shard
        while rows_left > 0:
            if rows_left >= 128 * r_main:
                plan.append((128, r_main))
                rows_left -= 128 * r_main
                continue
            for r in (r_main, 10, 5, 4, 2, 1):
                if rows_left % r == 0 and rows_left // r <= 128:
                    plan.append((rows_left // r, r))
                    rows_left = 0
                    break
            else:
                raise AssertionError(rows_left)
    tiles = []
    base = 0
    for parts, r in plan:
        tiles.append((base, parts, r))
        base += parts * r
    assert base == b_shard, (base, b_shard)
    return tiles


def _build_nc(b_shard: int, rows_per_part: int) -> bass.Bass:
    tiles = _plan_tiles(b_shard, rows_per_part)
    n_tiles = len(tiles)

    nc = bacc.Bacc("TRN2", target_bir_lowering=False, debug=False)
    inp = nc.dram_tensor("tensor", [b_shard, IN_C], F32, kind="ExternalInput")
    outp = nc.dram_tensor("out", [b_shard, TILT_C], F32, kind="ExternalOutput")

    # rsqrt epsilon bias: register as a const AP so activation(bias=...) works
    eps_t = nc.alloc_sbuf_tensor(f"const-float32-{RSQRT_EPS}", [128, 1], F32)
    nc.gpsimd.memset(eps_t.ap(), RSQRT_EPS)
    nc.const_aps.aps[(F32, RSQRT_EPS)] = eps_t.ap()
    nc.all_engine_barrier()

    with tile.TileContext(nc) as tc, ExitStack() as ctx:
        rawp = ctx.enter_context(tc.tile_pool(name="raw", bufs=2))
        pxyp = ctx.enter_context(tc.tile_pool(name="pxy", bufs=2))
        vxyp = ctx.enter_context(tc.tile_pool(name="vxy", bufs=4))
        sqp = ctx.enter_context(tc.tile_pool(name="sq", bufs=2))
        nsqp = ctx.enter_context(tc.tile_pool(name="nsq", bufs=2))
        rrp = ctx.enter_context(tc.tile_pool(name="rr", bufs=2))
        ttp = ctx.enter_context(tc.tile_pool(name="tt", bufs=5))

        st: dict = {}

        def st_load(t):
            base, parts, R = tiles[t]
            raw = rawp.tile([parts, R * IN_C], F32, tag="raw")
            src = inp[base:base + parts * R, :].rearrange(
                "(p r) c -> p (r c)", p=parts)
            nc.sync.dma_start(out=raw[:], in_=src.opt())
            st[t] = {"raw": raw}

        def st_gather(t):
            # gather (x,y) of each landmark into 4B-aligned packed fp16 pairs
            _, parts, R = tiles[t]
            raw = st[t]["raw"]
            raw4 = raw[:].rearrange("p (r l three) -> p r l three", l=NLM,
                                    three=3)[:, :, :, 0:2]
            pxy = pxyp.tile([parts, R * 2 * NLM], F16, tag="pxy")
            pxy4 = pxy[:].rearrange("p (r l two) -> p r l two", l=NLM, two=2)
            nc.scalar.activation(pxy4, raw4, AF.Copy)
            del st[t]["raw"]
            st[t]["pxy"] = pxy

        def st_sub(t):
            # pair differences: minuend dense fp16 step-1, subtrahend one
            # broadcast 32-bit (x,y) word -> DVE 2x packed mode
            _, parts, R = tiles[t]
            pxy = st[t]["pxy"]
            pxy4 = pxy[:].rearrange("p (r l two) -> p r l two", l=NLM, two=2)
            vxy = vxyp.tile([parts, R * TILT_C], F16, tag="vxy")
            vxy4 = vxy[:].rearrange("p (r q two) -> p r q two", q=NPAIR, two=2)
            pb = 0
            for i in range(NLM - 1):
                np_i = NLM - 1 - i
                minu = pxy4[:, :, i + 1:NLM, :]
                subt = pxy4[:, :, i:i + 1, :].broadcast_to(
                    (parts, R, np_i, 2))
                nc.vector.tensor_sub(vxy4[:, :, pb:pb + np_i, :], minu, subt)
                pb += np_i
            del st[t]["pxy"]
            st[t]["vxy"] = vxy

        def st_sq(t):
            _, parts, R = tiles[t]
            sq = sqp.tile([parts, R * TILT_C], F16, tag="sq")
            nc.scalar.activation(sq[:], st[t]["vxy"][:], AF.Square)
            st[t]["sq"] = sq

        def st_add(t):
            # duplicated-interleaved norm: nsqd[2q+c] = sq[2q] + sq[2q+1]
            _, parts, R = tiles[t]
            sqv = st[t]["sq"][:].rearrange("p (q two) -> p q two", two=2)
            nsqd = nsqp.tile([parts, R * TILT_C], F16, tag="nsq")
            nsqv = nsqd[:].rearrange("p (q two) -> p q two", two=2)
            nc.vector.tensor_add(nsqv, sqv, sqv[:, :, ::-1])
            del st[t]["sq"]
            st[t]["nsq"] = nsqd

        def st_rsq(t):
            _, parts, R = tiles[t]
            rrd = rrp.tile([parts, R * TILT_C], F16, tag="rr")
            nc.scalar.activation(rrd[:], st[t]["nsq"][:],
                                 AF.Abs_reciprocal_sqrt, bias=RSQRT_EPS)
            del st[t]["nsq"]
            st[t]["rr"] = rrd

        def st_mul(t):
            _, parts, R = tiles[t]
            tt = ttp.tile([parts, R * TILT_C], F16, tag="tt")
            nc.vector.tensor_mul(tt[:], st[t]["vxy"][:], st[t]["rr"][:])
            del st[t]["vxy"], st[t]["rr"]
            st[t]["tt"] = tt

        def st_out(t):
            # SWDGE cast-DMA: fp16 tilts in SBUF -> f32 rows in DRAM
            base, parts, R = tiles[t]
            dst = outp[base:base + parts * R, :].rearrange(
                "(p r) c -> p (r c)", p=parts)
            nc.gpsimd.dma_start(out=dst.opt(), in_=st[t]["tt"][:])
            del st[t]

        # tile t: load@t, gather@t (ACT tail), sub@t+1, sq@t+2, add@t+3,
        # rsq@t+4 (ACT head), mul+out@t+4. rsq(t) is ACT's first op of its
        # iteration and mul(t) is DVE's third, so the same-iteration
        # rsq->mul edge is hidden by queue position; every other dep is a
        # full iteration old. The shallow depth keeps the output-DMA
        # stream close behind compute (see _plan_tiles).
        for s in range(n_tiles + 5):
            if s < n_tiles:
                st_load(s)
            if 0 <= s - 1 < n_tiles:
                st_sub(s - 1)
            if 0 <= s - 3 < n_tiles:
                st_add(s - 3)
            if 0 <= s - 4 < n_tiles:
                st_rsq(s - 4)
                st_mul(s - 4)
                st_out(s - 4)
            if 0 <= s - 2 < n_tiles:
                st_sq(s - 2)
            if s < n_tiles:
                st_gather(s)

    nc.compile()
    return nc


_NC_CACHE: dict = {}


def _get_nc():
    key = (B_SHARD, ROWS_PER_PART)
    if key not in _NC_CACHE:
        _NC_CACHE[key] = _build_nc(B_SHARD, ROWS_PER_PART)
    return _NC_CACHE[key]


def kernel(tensor: np.ndarray) -> np.ndarray:
    tensor = np.ascontiguousarray(np.asarray(tensor, dtype=np.float32))
    assert tensor.shape == (B_FULL, IN_C), tensor.shape

    nc = _get_nc()
    in_maps = [
        {"tensor": tensor[c * B_SHARD:(c + 1) * B_SHARD]} for c in range(N_CORES)
    ]
    trace = os.environ.get("ANGULAR_TRACE", "0") == "1"
    res = run_bass_kernel_spmd(
        nc, in_maps, core_ids=list(range(N_CORES)), trace=trace
    )
    if trace:
        kernel.last_exec_time_ns = res.exec_time_ns
        kernel.last_results = res

    out = np.empty((B_FULL, OUT_C), dtype=np.float32)
    out[:, :IN_C] = tensor
    for c in range(N_CORES):
        out[c * B_SHARD:(c + 1) * B_SHARD, IN_C:] = res.results[c]["out"]
    return out


# revision 24
# speedup vs baseline: 1.0419x; 1.0099x over previous
"""AngularLayer Trainium2 kernel: [500000, 63] -> [500000, 483].

Per row: 21 (x,y) landmarks -> 210 ordered-pair unit direction vectors
(clipped x/y components), appended to the input row.

Sharded batch-parallel over 8 NeuronCores (62500 rows/core), SPMD one graph.

Device computes ONLY the 420 tilt columns; the 63 passthrough columns are a
verbatim copy of the input, assembled on host. Per tile [parts x R rows]:

- DMA-in (sync/HWDGE): dense f32 rows [parts, R*63].
- ACT: gather the 42 (x,y) coords per row into packed fp16 pairs `pxy`
  (4B-aligned -> enables DVE 2x_1P packed mode downstream).
- DVE: 20 ragged pair-difference subs (minuend dense step-1 fp16,
  subtrahend one broadcast 32-bit (x,y) word -> both qualify for 2x),
  swap-add for the duplicated interleaved norm, tilt multiply.
- ACT: Square + Abs_reciprocal_sqrt on the duplicated layout (1 elem/cyc
  engine). rsqrt bias 1e-7 guards exact fp16 landmark collisions (avoids
  inf -> NaN); also makes the clip unnecessary (|tilt| <= 1 + 2^-10).
- DMA-out (gpsimd/SWDGE): fp16 tilts cast to f32 rows IN THE DMA.
- GPSIMD runs no tensor ops at all: its SBUF port is shared with DVE and
  any sustained GPSIMD tensor op starves both engines (measured).

Depth-4 software pipeline (load/gather@t, sub@t+1, sq@t+2, add@t+3,
rsq+mul+out@t+4): every dep is a full iteration old except rsq->mul,
which is hidden by queue position (rsq is ACT's first op, mul DVE's
third). The shallow depth + medium-sized tail tiles keep the output-DMA
stream close behind compute (a deep pipeline or big tail tiles bunch the
last tiles' output DMAs after the final muls and pay a long post-compute
drain; tiny tail tiles crawl on per-DMA fixed costs).

Measured on HW (8 cores, SPMD): 418.5 us; rel_err 4.9e-4 (baseline
f32-subtract kernel: 814.5 us, rel_err 2.2e-3). DMA roofline ~340 us.
Taper tuning (measured): graded 21x16+[106,10]+4x14+11x8 = 418.5;
23x16+..14,14,12+9x8 = 422.8; 23x16+2x12+11x8 = 424.1; 25x16+10x8 =
427.8; [128,10]x8 tail = 431.8; 6x8 = 433.0. Tiny tiles ([128,4],
3360 B/partition) crawl at 52-130 GB/s in the cast-DMA - keep output
DMAs >= 6720 B/partition.
"""

import os
from contextlib import ExitStack

import numpy as np

import concourse.bass as bass
import concourse.mybir as mybir
import concourse.tile as tile
from concourse import bacc
from concourse.bass_utils import run_bass_kernel_spmd

F32 = mybir.dt.float32
F16 = mybir.dt.float16
AF = mybir.ActivationFunctionType
ALU = mybir.AluOpType

N_CORES = 8
B_FULL = 500000
B_SHARD = B_FULL // N_CORES  # 62500
NLM = 21
NPAIR = 210
IN_C = 63
TILT_C = 2 * NPAIR  # 420
OUT_C = 483

ROWS_PER_PART = int(os.environ.get("ANGULAR_R", "16"))
RSQRT_EPS = 1e-7


def _plan_tiles(b_shard: int, r_main: int):
    """[(base, parts, rows_per_part)] covering b_shard rows exactly.

    The tail tapers to small tiles: when loads stop, the last `depth`
    in-flight tiles' output DMAs bunch up after the final muls, so the
    post-compute drain is proportional to the byte size of the last few
    tiles, not their count.
    """
    plan = []
    if b_shard == 62500 and r_main == 16:
        # medium-sized tail tiles: small enough to limit the post-compute
        # output bunch, big enough that per-DMA fixed costs don't dominate
        plan = ([(128, 16)] * 19 + [(106, 10)] + [(128, 14)] * 8
                + [(128, 8)] * 8)
    else:
        rows_left = b_shard
        while rows_left > 0:
            if rows_left >= 128 * r_main:
                plan.append((128, r_main))
                rows_left -= 128 * r_main
                continue
            for r in (r_main, 10, 5, 4, 2, 1):
                if rows_left % r == 0 and rows_left // r <= 128:
                    plan.append((rows_left // r, r))
                    rows_left = 0
                    break
            else:
                raise AssertionError(rows_left)
    tiles = []
    base = 0
    for parts, r in plan:
        tiles.append((base, parts, r))
        base += parts * r
    assert base == b_shard, (base, b_shard)
    return tiles


def _build_nc(b_shard: int, rows_per_part: int) -> bass.Bass:
    tiles = _plan_tiles(b_shard, rows_per_part)
    n_tiles = len(tiles)

    nc = bacc.Bacc("TRN2", target_bir_lowering=False, debug=False)
    inp = nc.dram_tensor("tensor", [b_shard, IN_C], F32, kind="ExternalInput")
    outp = nc.dram_tensor("out", [b_shard, TILT_C], F32, kind="ExternalOutput")

    # rsqrt epsilon bias: register as a const AP so activation(bias=...) works
    eps_t = nc.alloc_sbuf_tensor(f"const-float32-{RSQRT_EPS}", [128, 1], F32)
    nc.gpsimd.memset(eps_t.ap(), RSQRT_EPS)
    nc.const_aps.aps[(F32, RSQRT_EPS)] = eps_t.ap()
    nc.all_engine_barrier()

    with tile.TileContext(nc) as tc, ExitStack() as ctx:
        rawp = ctx.enter_context(tc.tile_pool(name="raw", bufs=2))
        pxyp = ctx.enter_context(tc.tile_pool(name="pxy", bufs=2))
        vxyp = ctx.enter_context(tc.tile_pool(name="vxy", bufs=4))
        sqp = ctx.enter_context(tc.tile_pool(name="sq", bufs=2))
        nsqp = ctx.enter_context(tc.tile_pool(name="nsq", bufs=2))
        rrp = ctx.enter_context(tc.tile_pool(name="rr", bufs=2))
        ttp = ctx.enter_context(tc.tile_pool(name="tt", bufs=5))

        st: dict = {}

        def st_load(t):
            base, parts, R = tiles[t]
            raw = rawp.tile([parts, R * IN_C], F32, tag="raw")
            src = inp[base:base + parts * R, :].rearrange(
                "(p r) c -> p (r c)", p=parts)
            nc.sync.dma_start(out=raw[:], in_=src.opt())
            st[t] = {"raw": raw}

        def st_gather(t):
            # gather (x,y) of each landmark into 4B-aligned packed fp16 pairs
            _, parts, R = tiles[t]
            raw = st[t]["raw"]
            raw4 = raw[:].rearrange("p (r l three) -> p r l three", l=NLM,
                                    three=3)[:, :, :, 0:2]
            pxy = pxyp.tile([parts, R * 2 * NLM], F16, tag="pxy")
            pxy4 = pxy[:].rearrange("p (r l two) -> p r l two", l=NLM, two=2)
            nc.scalar.activation(pxy4, raw4, AF.Copy)
            del st[t]["raw"]
            st[t]["pxy"] = pxy

        def st_sub(t):
            # pair differences: minuend dense fp16 step-1, subtrahend one
            # broadcast 32-bit (x,y) word -> DVE 2x packed mode
            _, parts, R = tiles[t]
            pxy = st[t]["pxy"]
            pxy4 = pxy[:].rearrange("p (r l two) -> p r l two", l=NLM, two=2)
            vxy = vxyp.tile([parts, R * TILT_C], F16, tag="vxy")
            vxy4 = vxy[:].rearrange("p (r q two) -> p r q two", q=NPAIR, two=2)
            pb = 0
            for i in range(NLM - 1):
                np_i = NLM - 1 - i
                minu = pxy4[:, :, i + 1:NLM, :]
                subt = pxy4[:, :, i:i + 1, :].broadcast_to(
                    (parts, R, np_i, 2))
                nc.vector.tensor_sub(vxy4[:, :, pb:pb + np_i, :], minu, subt)
                pb += np_i
            del st[t]["pxy"]
            st[t]["vxy"] = vxy

        def st_sq(t):
            _, parts, R = tiles[t]
            sq = sqp.tile([parts, R * TILT_C], F16, tag="sq")
            nc.scalar.activation(sq[:], st[t]["vxy"][:], AF.Square)
            st[t]["sq"] = sq

        def st_add(t):
            # duplicated-interleaved norm: nsqd[2q+c] = sq[2q] + sq[2q+1]
            _, parts, R = tiles[t]
            sqv = st[t]["sq"][:].rearrange("p (q two) -> p q two", two=2)
            nsqd = nsqp.tile([parts, R * TILT_C], F16, tag="nsq")
            nsqv = nsqd[:].rearrange("p (q two) -> p q two", two=2)
            nc.vector.tensor_add(nsqv, sqv, sqv[:, :, ::-1])
            del st[t]["sq"]
            st[t]["nsq"] = nsqd

        def st_rsq(t):
            _, parts, R = tiles[t]
            rrd = rrp.tile([parts, R * TILT_C], F16, tag="rr")
            nc.scalar.activation(rrd[:], st[t]["nsq"][:],
                                 AF.Abs_reciprocal_sqrt, bias=RSQRT_EPS)
            del st[t]["nsq"]
            st[t]["rr"] = rrd

        def st_mul(t):
            _, parts, R = tiles[t]
            tt = ttp.tile([parts, R * TILT_C], F16, tag="tt")
            nc.vector.tensor_mul(tt[:], st[t]["vxy"][:], st[t]["rr"][:])
            del st[t]["vxy"], st[t]["rr"]
            st[t]["tt"] = tt

        def st_out(t):
            # SWDGE cast-DMA: fp16 tilts in SBUF -> f32 rows in DRAM
            base, parts, R = tiles[t]
            dst = outp[base:base + parts * R, :].rearrange(
                "(p r) c -> p (r c)", p=parts)
            nc.gpsimd.dma_start(out=dst.opt(), in_=st[t]["tt"][:])
            del st[t]

        # tile t: load@t, gather@t (ACT tail), sub@t+1, sq@t+2, add@t+3,
        # rsq@t+4 (ACT head), mul+out@t+4. rsq(t) is ACT's first op of its
        # iteration and mul(t) is DVE's third, so the same-iteration
        # rsq->mul edge is hidden by queue position; every other dep is a
        # full iteration old. The shallow depth keeps the output-DMA
        # stream close behind compute (see _plan_tiles).
        for s in range(n_tiles + 5):
            if s < n_tiles:
                st_load(s)
            if 0 <= s - 1 < n_tiles:
                st_sub(s - 1)
            if 0 <= s - 3 < n_tiles:
                st_add(s - 3)
            if 0 <= s - 4 < n_tiles:
                st_rsq(s - 4)
                st_mul(s - 4)
                st_out(s - 4)
            if 0 <= s - 2 < n_tiles:
                st_sq(s - 2)
            if s < n_tiles:
                st_gather(s)

    nc.compile()
    return nc


_NC_CACHE: dict = {}


def _get_nc():
    key = (B_SHARD, ROWS_PER_PART)
    if key not in _NC_CACHE:
        _NC_CACHE[key] = _build_nc(B_SHARD, ROWS_PER_PART)
    return _NC_CACHE[key]


def kernel(tensor: np.ndarray) -> np.ndarray:
    tensor = np.ascontiguousarray(np.asarray(tensor, dtype=np.float32))
    assert tensor.shape == (B_FULL, IN_C), tensor.shape

    nc = _get_nc()
    in_maps = [
        {"tensor": tensor[c * B_SHARD:(c + 1) * B_SHARD]} for c in range(N_CORES)
    ]
    trace = os.environ.get("ANGULAR_TRACE", "0") == "1"
    res = run_bass_kernel_spmd(
        nc, in_maps, core_ids=list(range(N_CORES)), trace=trace
    )
    if trace:
        kernel.last_exec_time_ns = res.exec_time_ns
        kernel.last_results = res

    out = np.empty((B_FULL, OUT_C), dtype=np.float32)
    out[:, :IN_C] = tensor
    for c in range(N_CORES):
        out[c * B_SHARD:(c + 1) * B_SHARD, IN_C:] = res.results[c]["out"]
    return out


# revision 25
# speedup vs baseline: 1.0420x; 1.0001x over previous
"""AngularLayer Trainium2 kernel: [500000, 63] -> [500000, 483].

Per row: 21 (x,y) landmarks -> 210 ordered-pair unit direction vectors
(clipped x/y components), appended to the input row.

Sharded batch-parallel over 8 NeuronCores (62500 rows/core), SPMD one graph.

Device computes ONLY the 420 tilt columns; the 63 passthrough columns are a
verbatim copy of the input, assembled on host. Per tile [parts x R rows]:

- DMA-in (sync/HWDGE): dense f32 rows [parts, R*63].
- ACT: gather the 42 (x,y) coords per row into packed fp16 pairs `pxy`
  (4B-aligned -> enables DVE 2x_1P packed mode downstream).
- DVE: 20 ragged pair-difference subs (minuend dense step-1 fp16,
  subtrahend one broadcast 32-bit (x,y) word -> both qualify for 2x),
  swap-add for the duplicated interleaved norm, tilt multiply.
- ACT: Square + Abs_reciprocal_sqrt on the duplicated layout (1 elem/cyc
  engine). rsqrt bias 1e-7 guards exact fp16 landmark collisions (avoids
  inf -> NaN); also makes the clip unnecessary (|tilt| <= 1 + 2^-10).
- DMA-out (gpsimd/SWDGE): fp16 tilts cast to f32 rows IN THE DMA.
- GPSIMD runs no tensor ops at all: its SBUF port is shared with DVE and
  any sustained GPSIMD tensor op starves both engines (measured).

Depth-4 software pipeline (load/gather@t, sub@t+1, sq@t+2, add@t+3,
rsq+mul+out@t+4): every dep is a full iteration old except rsq->mul,
which is hidden by queue position (rsq is ACT's first op, mul DVE's
third). The shallow depth + medium-sized tail tiles keep the output-DMA
stream close behind compute (a deep pipeline or big tail tiles bunch the
last tiles' output DMAs after the final muls and pay a long post-compute
drain; tiny tail tiles crawl on per-DMA fixed costs).

Measured on HW (8 cores, SPMD): 418.5 us; rel_err 4.9e-4 (baseline
f32-subtract kernel: 814.5 us, rel_err 2.2e-3). DMA roofline ~340 us.
Taper tuning (measured): graded 21x16+[106,10]+4x14+11x8 = 418.5;
23x16+..14,14,12+9x8 = 422.8; 23x16+2x12+11x8 = 424.1; 25x16+10x8 =
427.8; [128,10]x8 tail = 431.8; 6x8 = 433.0. Tiny tiles ([128,4],
3360 B/partition) crawl at 52-130 GB/s in the cast-DMA - keep output
DMAs >= 6720 B/partition.
"""

import os
from contextlib import ExitStack

import numpy as np

import concourse.bass as bass
import concourse.mybir as mybir
import concourse.tile as tile
from concourse import bacc
from concourse.bass_utils import run_bass_kernel_spmd

F32 = mybir.dt.float32
F16 = mybir.dt.float16
AF = mybir.ActivationFunctionType
ALU = mybir.AluOpType

N_CORES = 8
B_FULL = 500000
B_SHARD = B_FULL // N_CORES  # 62500
NLM = 21
NPAIR = 210
IN_C = 63
TILT_C = 2 * NPAIR  # 420
OUT_C = 483

ROWS_PER_PART = int(os.environ.get("ANGULAR_R", "16"))
RSQRT_EPS = 1e-7


def _plan_tiles(b_shard: int, r_main: int):
    """[(base, parts, rows_per_part)] covering b_shard rows exactly.

    The tail tapers to small tiles: when loads stop, the last `depth`
    in-flight tiles' output DMAs bunch up after the final muls, so the
    post-compute drain is proportional to the byte size of the last few
    tiles, not their count.
    """
    plan = []
    if b_shard == 62500 and r_main == 16:
        # medium-sized tail tiles: small enough to limit the post-compute
        # output bunch, big enough that per-DMA fixed costs don't dominate
        plan = ([(128, 16)] * 16 + [(106, 10)] + [(128, 14)] * 12
                + [(128, 8)] * 7)
    else:
        rows_left = b_shard
        while rows_left > 0:
            if rows_left >= 128 * r_main:
                plan.append((128, r_main))
                rows_left -= 128 * r_main
                continue
            for r in (r_main, 10, 5, 4, 2, 1):
                if rows_left % r == 0 and rows_left // r <= 128:
                    plan.append((rows_left // r, r))
                    rows_left = 0
                    break
            else:
                raise AssertionError(rows_left)
    tiles = []
    base = 0
    for parts, r in plan:
        tiles.append((base, parts, r))
        base += parts * r
    assert base == b_shard, (base, b_shard)
    return tiles


def _build_nc(b_shard: int, rows_per_part: int) -> bass.Bass:
    tiles = _plan_tiles(b_shard, rows_per_part)
    n_tiles = len(tiles)

    nc = bacc.Bacc("TRN2", target_bir_lowering=False, debug=False)
    inp = nc.dram_tensor("tensor", [b_shard, IN_C], F32, kind="ExternalInput")
    outp = nc.dram_tensor("out", [b_shard, TILT_C], F32, kind="ExternalOutput")

    # rsqrt epsilon bias: register as a const AP so activation(bias=...) works
    eps_t = nc.alloc_sbuf_tensor(f"const-float32-{RSQRT_EPS}", [128, 1], F32)
    nc.gpsimd.memset(eps_t.ap(), RSQRT_EPS)
    nc.const_aps.aps[(F32, RSQRT_EPS)] = eps_t.ap()
    nc.all_engine_barrier()

    with tile.TileContext(nc) as tc, ExitStack() as ctx:
        rawp = ctx.enter_context(tc.tile_pool(name="raw", bufs=2))
        pxyp = ctx.enter_context(tc.tile_pool(name="pxy", bufs=2))
        vxyp = ctx.enter_context(tc.tile_pool(name="vxy", bufs=4))
        sqp = ctx.enter_context(tc.tile_pool(name="sq", bufs=2))
        nsqp = ctx.enter_context(tc.tile_pool(name="nsq", bufs=2))
        rrp = ctx.enter_context(tc.tile_pool(name="rr", bufs=2))
        ttp = ctx.enter_context(tc.tile_pool(name="tt", bufs=5))

        st: dict = {}

        def st_load(t):
            base, parts, R = tiles[t]
            raw = rawp.tile([parts, R * IN_C], F32, tag="raw")
            src = inp[base:base + parts * R, :].rearrange(
                "(p r) c -> p (r c)", p=parts)
            nc.sync.dma_start(out=raw[:], in_=src.opt())
            st[t] = {"raw": raw}

        def st_gather(t):
            # gather (x,y) of each landmark into 4B-aligned packed fp16 pairs
            _, parts, R = tiles[t]
            raw = st[t]["raw"]
            raw4 = raw[:].rearrange("p (r l three) -> p r l three", l=NLM,
                                    three=3)[:, :, :, 0:2]
            pxy = pxyp.tile([parts, R * 2 * NLM], F16, tag="pxy")
            pxy4 = pxy[:].rearrange("p (r l two) -> p r l two", l=NLM, two=2)
            nc.scalar.activation(pxy4, raw4, AF.Copy)
            del st[t]["raw"]
            st[t]["pxy"] = pxy

        def st_sub(t):
            # pair differences: minuend dense fp16 step-1, subtrahend one
            # broadcast 32-bit (x,y) word -> DVE 2x packed mode
            _, parts, R = tiles[t]
            pxy = st[t]["pxy"]
            pxy4 = pxy[:].rearrange("p (r l two) -> p r l two", l=NLM, two=2)
            vxy = vxyp.tile([parts, R * TILT_C], F16, tag="vxy")
            vxy4 = vxy[:].rearrange("p (r q two) -> p r q two", q=NPAIR, two=2)
            pb = 0
            for i in range(NLM - 1):
                np_i = NLM - 1 - i
                minu = pxy4[:, :, i + 1:NLM, :]
                subt = pxy4[:, :, i:i + 1, :].broadcast_to(
                    (parts, R, np_i, 2))
                nc.vector.tensor_sub(vxy4[:, :, pb:pb + np_i, :], minu, subt)
                pb += np_i
            del st[t]["pxy"]
            st[t]["vxy"] = vxy

        def st_sq(t):
            _, parts, R = tiles[t]
            sq = sqp.tile([parts, R * TILT_C], F16, tag="sq")
            nc.scalar.activation(sq[:], st[t]["vxy"][:], AF.Square)
            st[t]["sq"] = sq

        def st_add(t):
            # duplicated-interleaved norm: nsqd[2q+c] = sq[2q] + sq[2q+1]
            _, parts, R = tiles[t]
            sqv = st[t]["sq"][:].rearrange("p (q two) -> p q two", two=2)
            nsqd = nsqp.tile([parts, R * TILT_C], F16, tag="nsq")
            nsqv = nsqd[:].rearrange("p (q two) -> p q two", two=2)
            nc.vector.tensor_add(nsqv, sqv, sqv[:, :, ::-1])
            del st[t]["sq"]
            st[t]["nsq"] = nsqd

        def st_rsq(t):
            _, parts, R = tiles[t]
            rrd = rrp.tile([parts, R * TILT_C], F16, tag="rr")
            nc.scalar.activation(rrd[:], st[t]["nsq"][:],
                                 AF.Abs_reciprocal_sqrt, bias=RSQRT_EPS)
            del st[t]["nsq"]
            st[t]["rr"] = rrd

        def st_mul(t):
            _, parts, R = tiles[t]
            tt = ttp.tile([parts, R * TILT_C], F16, tag="tt")
            nc.vector.tensor_mul(tt[:], st[t]["vxy"][:], st[t]["rr"][:])
            del st[t]["vxy"], st[t]["rr"]
            st[t]["tt"] = tt

        def st_out(t):
            # SWDGE cast-DMA: fp16 tilts in SBUF -> f32 rows in DRAM
            base, parts, R = tiles[t]
            dst = outp[base:base + parts * R, :].rearrange(
                "(p r) c -> p (r c)", p=parts)
            nc.gpsimd.dma_start(out=dst.opt(), in_=st[t]["tt"][:])
            del st[t]

        # tile t: load@t, gather@t (ACT tail), sub@t+1, sq@t+2, add@t+3,
        # rsq@t+4 (ACT head), mul+out@t+4. rsq(t) is ACT's first op of its
        # iteration and mul(t) is DVE's third, so the same-iteration
        # rsq->mul edge is hidden by queue position; every other dep is a
        # full iteration old. The shallow depth keeps the output-DMA
        # stream close behind compute (see _plan_tiles).
        for s in range(n_tiles + 5):
            if s < n_tiles:
                st_load(s)
            if 0 <= s - 1 < n_tiles:
                st_sub(s - 1)
            if 0 <= s - 3 < n_tiles:
                st_add(s - 3)
            if 0 <= s - 4 < n_tiles:
                st_rsq(s - 4)
                st_mul(s - 4)
                st_out(s - 4)
            if 0 <= s - 2 < n_tiles:
                st_sq(s - 2)
            if s < n_tiles:
                st_gather(s)

    nc.compile()
    return nc


_NC_CACHE: dict = {}


def _get_nc():
    key = (B_SHARD, ROWS_PER_PART)
    if key not in _NC_CACHE:
        _NC_CACHE[key] = _build_nc(B_SHARD, ROWS_PER_PART)
    return _NC_CACHE[key]


def kernel(tensor: np.ndarray) -> np.ndarray:
    tensor = np.ascontiguousarray(np.asarray(tensor, dtype=np.float32))
    assert tensor.shape == (B_FULL, IN_C), tensor.shape

    nc = _get_nc()
    in_maps = [
        {"tensor": tensor[c * B_SHARD:(c + 1) * B_SHARD]} for c in range(N_CORES)
    ]
    trace = os.environ.get("ANGULAR_TRACE", "0") == "1"
    res = run_bass_kernel_spmd(
        nc, in_maps, core_ids=list(range(N_CORES)), trace=trace
    )
    if trace:
        kernel.last_exec_time_ns = res.exec_time_ns
        kernel.last_results = res

    out = np.empty((B_FULL, OUT_C), dtype=np.float32)
    out[:, :IN_C] = tensor
    for c in range(N_CORES):
        out[c * B_SHARD:(c + 1) * B_SHARD, IN_C:] = res.results[c]["out"]
    return out


# revision 27
# speedup vs baseline: 1.0517x; 1.0093x over previous
"""AngularLayer Trainium2 kernel: [500000, 63] -> [500000, 483].

Per row: 21 (x,y) landmarks -> 210 ordered-pair unit direction vectors
(clipped x/y components), appended to the input row.

Sharded batch-parallel over 8 NeuronCores (62500 rows/core), SPMD one graph.

Device computes ONLY the 420 tilt columns; the 63 passthrough columns are a
verbatim copy of the input, assembled on host. Per tile [parts x R rows]:

- DMA-in (sync/HWDGE): dense f32 rows [parts, R*63].
- ACT: gather the 42 (x,y) coords per row into packed fp16 pairs `pxy`
  (4B-aligned -> enables DVE 2x_1P packed mode downstream).
- DVE: 20 ragged pair-difference subs (minuend dense step-1 fp16,
  subtrahend one broadcast 32-bit (x,y) word -> both qualify for 2x),
  swap-add for the duplicated interleaved norm, tilt multiply.
- ACT: Square + Abs_reciprocal_sqrt on the duplicated layout (1 elem/cyc
  engine). rsqrt bias 1e-7 guards exact fp16 landmark collisions (avoids
  inf -> NaN); also makes the clip unnecessary (|tilt| <= 1 + 2^-10).
- DMA-out (gpsimd/SWDGE): fp16 tilts cast to f32 rows IN THE DMA.
- GPSIMD runs no tensor ops at all: its SBUF port is shared with DVE and
  any sustained GPSIMD tensor op starves both engines (measured).

Depth-4 software pipeline (load/gather@t, sub@t+1, sq@t+2, add@t+3,
rsq+mul+out@t+4): every dep is a full iteration old except rsq->mul,
which is hidden by queue position (rsq is ACT's first op, mul DVE's
third). The shallow depth + medium-sized tail tiles keep the output-DMA
stream close behind compute (a deep pipeline or big tail tiles bunch the
last tiles' output DMAs after the final muls and pay a long post-compute
drain; tiny tail tiles crawl on per-DMA fixed costs).

Measured on HW (8 cores, SPMD): 414.4 us; rel_err 4.9e-4 (baseline
f32-subtract kernel: 814.5 us, rel_err 2.2e-3). DMA roofline ~340 us.
Taper ladder (measured, monotone until converged): 16x16+12x14+7x8 =
414.4 ~= 19x16+8x14+8x8 = 414.4 < 21x16+4x14+11x8 = 418.5 <
23x16+2x14+12+9x8 = 422.8 < 23x16+2x12+11x8 = 424.1 < 25x16+10x8 =
427.8 < 6x8 = 433.0 (each with the [106,10] remainder after the 16s).
Tiny tiles ([128,4], 3360 B/partition) crawl at 52-130 GB/s in the
cast-DMA - keep output DMAs >= 6720 B/partition. At convergence DVE is
continuous 12->401 us with 7 us of holes; the rest is NEFF preamble
(~12 us) and final drain/teardown (~19 us).
"""

import os
from contextlib import ExitStack

import numpy as np

import concourse.bass as bass
import concourse.mybir as mybir
import concourse.tile as tile
from concourse import bacc
from concourse.bass_utils import run_bass_kernel_spmd

F32 = mybir.dt.float32
F16 = mybir.dt.float16
AF = mybir.ActivationFunctionType
ALU = mybir.AluOpType

N_CORES = 8
B_FULL = 500000
B_SHARD = B_FULL // N_CORES  # 62500
NLM = 21
NPAIR = 210
IN_C = 63
TILT_C = 2 * NPAIR  # 420
OUT_C = 483

ROWS_PER_PART = int(os.environ.get("ANGULAR_R", "16"))
RSQRT_EPS = 1e-7


def _plan_tiles(b_shard: int, r_main: int):
    """[(base, parts, rows_per_part)] covering b_shard rows exactly.

    The tail tapers to small tiles: when loads stop, the last `depth`
    in-flight tiles' output DMAs bunch up after the final muls, so the
    post-compute drain is proportional to the byte size of the last few
    tiles, not their count.
    """
    plan = []
    if b_shard == 62500 and r_main == 16:
        # medium-sized tail tiles: small enough to limit the post-compute
        # output bunch, big enough that per-DMA fixed costs don't dominate
        plan = ([(128, 16)] * 16 + [(106, 10)] + [(128, 14)] * 12
                + [(128, 8)] * 7)
    else:
        rows_left = b_shard
        while rows_left > 0:
            if rows_left >= 128 * r_main:
                plan.append((128, r_main))
                rows_left -= 128 * r_main
                continue
            for r in (r_main, 10, 5, 4, 2, 1):
                if rows_left % r == 0 and rows_left // r <= 128:
                    plan.append((rows_left // r, r))
                    rows_left = 0
                    break
            else:
                raise AssertionError(rows_left)
    tiles = []
    base = 0
    for parts, r in plan:
        tiles.append((base, parts, r))
        base += parts * r
    assert base == b_shard, (base, b_shard)
    return tiles


def _build_nc(b_shard: int, rows_per_part: int) -> bass.Bass:
    tiles = _plan_tiles(b_shard, rows_per_part)
    n_tiles = len(tiles)

    nc = bacc.Bacc("TRN2", target_bir_lowering=False, debug=False)
    inp = nc.dram_tensor("tensor", [b_shard, IN_C], F32, kind="ExternalInput")
    outp = nc.dram_tensor("out", [b_shard, TILT_C], F32, kind="ExternalOutput")

    # rsqrt epsilon bias: register as a const AP so activation(bias=...) works
    eps_t = nc.alloc_sbuf_tensor(f"const-float32-{RSQRT_EPS}", [128, 1], F32)
    nc.gpsimd.memset(eps_t.ap(), RSQRT_EPS)
    nc.const_aps.aps[(F32, RSQRT_EPS)] = eps_t.ap()
    nc.all_engine_barrier()

    with tile.TileContext(nc) as tc, ExitStack() as ctx:
        rawp = ctx.enter_context(tc.tile_pool(name="raw", bufs=2))
        pxyp = ctx.enter_context(tc.tile_pool(name="pxy", bufs=2))
        vxyp = ctx.enter_context(tc.tile_pool(name="vxy", bufs=4))
        sqp = ctx.enter_context(tc.tile_pool(name="sq", bufs=2))
        nsqp = ctx.enter_context(tc.tile_pool(name="nsq", bufs=2))
        rrp = ctx.enter_context(tc.tile_pool(name="rr", bufs=2))
        ttp = ctx.enter_context(tc.tile_pool(name="tt", bufs=5))

        st: dict = {}

        def st_load(t):
            base, parts, R = tiles[t]
            raw = rawp.tile([parts, R * IN_C], F32, tag="raw")
            src = inp[base:base + parts * R, :].rearrange(
                "(p r) c -> p (r c)", p=parts)
            nc.sync.dma_start(out=raw[:], in_=src.opt())
            st[t] = {"raw": raw}

        def st_gather(t):
            # gather (x,y) of each landmark into 4B-aligned packed fp16 pairs
            _, parts, R = tiles[t]
            raw = st[t]["raw"]
            raw4 = raw[:].rearrange("p (r l three) -> p r l three", l=NLM,
                                    three=3)[:, :, :, 0:2]
            pxy = pxyp.tile([parts, R * 2 * NLM], F16, tag="pxy")
            pxy4 = pxy[:].rearrange("p (r l two) -> p r l two", l=NLM, two=2)
            nc.scalar.activation(pxy4, raw4, AF.Copy)
            del st[t]["raw"]
            st[t]["pxy"] = pxy

        def st_sub(t):
            # pair differences: minuend dense fp16 step-1, subtrahend one
            # broadcast 32-bit (x,y) word -> DVE 2x packed mode
            _, parts, R = tiles[t]
            pxy = st[t]["pxy"]
            pxy4 = pxy[:].rearrange("p (r l two) -> p r l two", l=NLM, two=2)
            vxy = vxyp.tile([parts, R * TILT_C], F16, tag="vxy")
            vxy4 = vxy[:].rearrange("p (r q two) -> p r q two", q=NPAIR, two=2)
            pb = 0
            for i in range(NLM - 1):
                np_i = NLM - 1 - i
                minu = pxy4[:, :, i + 1:NLM, :]
                subt = pxy4[:, :, i:i + 1, :].broadcast_to(
                    (parts, R, np_i, 2))
                nc.vector.tensor_sub(vxy4[:, :, pb:pb + np_i, :], minu, subt)
                pb += np_i
            del st[t]["pxy"]
            st[t]["vxy"] = vxy

        def st_sq(t):
            _, parts, R = tiles[t]
            sq = sqp.tile([parts, R * TILT_C], F16, tag="sq")
            nc.scalar.activation(sq[:], st[t]["vxy"][:], AF.Square)
            st[t]["sq"] = sq

        def st_add(t):
            # duplicated-interleaved norm: nsqd[2q+c] = sq[2q] + sq[2q+1]
            _, parts, R = tiles[t]
            sqv = st[t]["sq"][:].rearrange("p (q two) -> p q two", two=2)
            nsqd = nsqp.tile([parts, R * TILT_C], F16, tag="nsq")
            nsqv = nsqd[:].rearrange("p (q two) -> p q two", two=2)
            nc.vector.tensor_add(nsqv, sqv, sqv[:, :, ::-1])
            del st[t]["sq"]
            st[t]["nsq"] = nsqd

        def st_rsq(t):
            _, parts, R = tiles[t]
            rrd = rrp.tile([parts, R * TILT_C], F16, tag="rr")
            nc.scalar.activation(rrd[:], st[t]["nsq"][:],
                                 AF.Abs_reciprocal_sqrt, bias=RSQRT_EPS)
            del st[t]["nsq"]
            st[t]["rr"] = rrd

        def st_mul(t):
            _, parts, R = tiles[t]
            tt = ttp.tile([parts, R * TILT_C], F16, tag="tt")
            nc.vector.tensor_mul(tt[:], st[t]["vxy"][:], st[t]["rr"][:])
            del st[t]["vxy"], st[t]["rr"]
            st[t]["tt"] = tt

        def st_out(t):
            # SWDGE cast-DMA: fp16 tilts in SBUF -> f32 rows in DRAM
            base, parts, R = tiles[t]
            dst = outp[base:base + parts * R, :].rearrange(
                "(p r) c -> p (r c)", p=parts)
            nc.gpsimd.dma_start(out=dst.opt(), in_=st[t]["tt"][:])
            del st[t]

        # tile t: load@t, gather@t (ACT tail), sub@t+1, sq@t+2, add@t+3,
        # rsq@t+4 (ACT head), mul+out@t+4. rsq(t) is ACT's first op of its
        # iteration and mul(t) is DVE's third, so the same-iteration
        # rsq->mul edge is hidden by queue position; every other dep is a
        # full iteration old. The shallow depth keeps the output-DMA
        # stream close behind compute (see _plan_tiles).
        for s in range(n_tiles + 5):
            if s < n_tiles:
                st_load(s)
            if 0 <= s - 1 < n_tiles:
                st_sub(s - 1)
            if 0 <= s - 3 < n_tiles:
                st_add(s - 3)
            if 0 <= s - 4 < n_tiles:
                st_rsq(s - 4)
                st_mul(s - 4)
                st_out(s - 4)
            if s < n_tiles:
                st_gather(s)
            if 0 <= s - 2 < n_tiles:
                st_sq(s - 2)

    nc.compile()
    return nc


_NC_CACHE: dict = {}


def _get_nc():
    key = (B_SHARD, ROWS_PER_PART)
    if key not in _NC_CACHE:
        _NC_CACHE[key] = _build_nc(B_SHARD, ROWS_PER_PART)
    return _NC_CACHE[key]


def kernel(tensor: np.ndarray) -> np.ndarray:
    tensor = np.ascontiguousarray(np.asarray(tensor, dtype=np.float32))
    assert tensor.shape == (B_FULL, IN_C), tensor.shape

    nc = _get_nc()
    in_maps = [
        {"tensor": tensor[c * B_SHARD:(c + 1) * B_SHARD]} for c in range(N_CORES)
    ]
    trace = os.environ.get("ANGULAR_TRACE", "0") == "1"
    res = run_bass_kernel_spmd(
        nc, in_maps, core_ids=list(range(N_CORES)), trace=trace
    )
    if trace:
        kernel.last_exec_time_ns = res.exec_time_ns
        kernel.last_results = res

    out = np.empty((B_FULL, OUT_C), dtype=np.float32)
    out[:, :IN_C] = tensor
    for c in range(N_CORES):
        out[c * B_SHARD:(c + 1) * B_SHARD, IN_C:] = res.results[c]["out"]
    return out
